# revision 1
# baseline (speedup 1.0000x reference)
"""Trainium2 Bass kernel for the 5-head detection tower (nn_DFD_10849087390476).

Network (per head h of 5): 1x1-conv tower on x [B,64,H,W]:
    h1 = relu(Win x + bin)
    h2 = h1 + relu(Wh0 h1 + bh0)
    h3 = h2 + relu(Wh1 h2 + bh1)
    out_h = Wout h3 + bout
Output = concat over heads: channels [cls 81, obj 2, box 4, pos 64, ins 128] = 279.

Strategy: data-parallel over (batch, H/2) -> 8 shards of 32768 pixels.
Per core the pixel set is split in two 16384-px groups (A on SBUF partitions
0-63, B on 64-127). Heads are paired (cls+obj, box+pos) with block-diagonal
128x128 stationaries so every matmul uses the full partition dim; the ins head
rides the A/B stacking instead. All matmuls run in float32r (TF32-like, 1
cycle/row at free-dim 512). Residuals are fused on the vector engine as
max(psum,0)+h via scalar_tensor_tensor; in-layer bias comes free via the
scalar-engine activation bias, out-layer bias via activation Identity+bias.
"""
import numpy as np

from concourse import bacc, tile
import concourse.mybir as mybir
from concourse.bass_utils import run_bass_kernel_spmd

F32 = mybir.dt.float32
F32R = mybir.dt.float32r
AF = mybir.ActivationFunctionType
ALU = mybir.AluOpType

B, C, H, W = 4, 64, 256, 256
NCORES = 8
NPX = (B * H * W) // NCORES          # 32768 pixels per core
NG = NPX // 2                        # 16384 per group (A/B)
T = 512                              # pixels per matmul tile
NT = NG // T                         # 32 pair-tiles per core
OD = 279                             # output channels

_last_results = None                 # test.py reads exec_time_ns from here
_cache = {}


def _bd(a, b):
    """block-diag of two 2D arrays."""
    out = np.zeros((a.shape[0] + b.shape[0], a.shape[1] + b.shape[1]), np.float32)
    out[:a.shape[0], :a.shape[1]] = a
    out[a.shape[0]:, a.shape[1]:] = b
    return out


def _build(fast: bool):
    nc = bacc.Bacc("TRN2", target_bir_lowering=False, debug=False)

    xs_d = nc.dram_tensor("xs", [128, NG], F32, kind="ExternalInput")
    w_names = ["sin_co", "sin_bp", "sin_ins",
               "sl1_co", "sl1_bp", "sl1_ins",
               "sl2_co", "sl2_bp", "sl2_ins"]
    w_d = {n: nc.dram_tensor(n, [128, 128], F32, kind="ExternalInput")
           for n in w_names}
    w_d["sout_co"] = nc.dram_tensor("sout_co", [128, 83], F32, kind="ExternalInput")
    w_d["sout_bp"] = nc.dram_tensor("sout_bp", [128, 68], F32, kind="ExternalInput")
    w_d["sout_ins"] = nc.dram_tensor("sout_ins", [128, 128], F32, kind="ExternalInput")
    bin_d = {s: nc.dram_tensor(f"bin_{s}", [128, 1], F32, kind="ExternalInput")
             for s in ("co", "bp", "ins")}
    bh_d = {(l, s): nc.dram_tensor(f"bh{l}_{s}", [128, 1], F32, kind="ExternalInput")
            for l in (1, 2) for s in ("co", "bp", "ins")}
    bout_d = {"co": nc.dram_tensor("bout_co", [83, 1], F32, kind="ExternalInput"),
              "bp": nc.dram_tensor("bout_bp", [68, 1], F32, kind="ExternalInput"),
              "ins": nc.dram_tensor("bout_ins", [128, 1], F32, kind="ExternalInput")}
    out_d = nc.dram_tensor("out", [OD, NPX], F32, kind="ExternalOutput")

    # output channel ranges per section
    OCH = {"co": (0, 83), "bp": (83, 151), "ins": (151, 279)}

    with tile.TileContext(nc) as tc:
        with tc.tile_pool(name="const", bufs=1) as cpool, \
             tc.tile_pool(name="xp", bufs=4) as xpool, \
             tc.tile_pool(name="hp", bufs=4) as hpool, \
             tc.tile_pool(name="op", bufs=3) as opool, \
             tc.tile_pool(name="ps", bufs=2, space="PSUM") as pspool, \
             tc.tile_pool(name="pso", bufs=2, space="PSUM") as psopool:

            wt = {}
            for n, d in w_d.items():
                t_ = cpool.tile([128, d.shape[1]], F32R, tag=n)
                nc.sync.dma_start(out=t_[:], in_=d.ap().bitcast(F32R))
                wt[n] = t_
            bin_t = {}
            for s, d in bin_d.items():
                t_ = cpool.tile([128, 1], F32, tag=f"bin{s}")
                nc.sync.dma_start(out=t_[:], in_=d.ap())
                bin_t[s] = t_
            bh_t = {}
            if not fast:
                for (l, s), d in bh_d.items():
                    t_ = cpool.tile([128, 1], F32, tag=f"bh{l}{s}")
                    nc.sync.dma_start(out=t_[:], in_=d.ap())
                    bh_t[(l, s)] = t_
            bout_t = {}
            for s, d in bout_d.items():
                t_ = cpool.tile([d.shape[0], 1], F32, tag=f"bout{s}")
                nc.sync.dma_start(out=t_[:], in_=d.ap())
                bout_t[s] = t_

            def residual(h_prev, psum, l, s):
                """h_next = h_prev + relu(psum + bh)."""
                P, FD = h_prev.shape[0], h_prev.shape[1]
                h_next = hpool.tile([P, FD], F32R, tag=f"h{s}")
                if fast:
                    nc.vector.scalar_tensor_tensor(
                        h_next[:], psum[:], 0.0, h_prev[:], ALU.max, ALU.add)
                else:
                    r = hpool.tile([P, FD], F32, tag=f"r{s}")
                    nc.scalar.activation(r[:], psum[:], AF.Relu,
                                         bias=bh_t[(l, s)][:], scale=1.0)
                    nc.vector.tensor_add(h_next[:], h_prev[:], r[:])
                return h_next

            for g in range(NT):
                c0 = g * T
                x_t = xpool.tile([128, T], F32R, tag="x")
                nc.sync.dma_start(out=x_t[:], in_=xs_d.ap()[:, c0:c0 + T].bitcast(F32R))

                for s in ("co", "bp", "ins"):
                    mo = w_d["sout_" + s].shape[1]   # 83 / 68 / 128
                    # ---- in-proj ----
                    if s == "ins":
                        ps_in = pspool.tile([128, T], F32, tag="ps")
                        nc.tensor.matmul(ps_in[:], wt["sin_ins"][:], x_t[:],
                                         start=True, stop=True)
                        h1 = hpool.tile([128, T], F32R, tag="hins")
                        nc.scalar.activation(h1[:], ps_in[:], AF.Relu,
                                             bias=bin_t[s][:], scale=1.0)
                    else:
                        ps_in = pspool.tile([128, 2 * T], F32, tag="ps")
                        nc.tensor.matmul(ps_in[:, 0:T], wt["sin_" + s][0:64, :],
                                         x_t[0:64, :], start=True, stop=True)
                        nc.tensor.matmul(ps_in[:, T:2 * T], wt["sin_" + s][64:128, :],
                                         x_t[64:128, :], start=True, stop=True)
                        h1 = hpool.tile([128, 2 * T], F32R, tag="h" + s)
                        nc.scalar.activation(h1[:], ps_in[:], AF.Relu,
                                             bias=bin_t[s][:], scale=1.0)

                    # ---- hidden layers ----
                    h = h1
                    for l, wname in ((1, f"sl1_{s}"), (2, f"sl2_{s}")):
                        FD = h.shape[1]
                        ps_l = pspool.tile([128, FD], F32, tag="ps")
                        for k in range(FD // T):
                            nc.tensor.matmul(ps_l[:, k * T:(k + 1) * T], wt[wname][:],
                                             h[:, k * T:(k + 1) * T],
                                             start=True, stop=True)
                        h = residual(h, ps_l, l, s)

                    # ---- out-proj ----
                    ps_o = psopool.tile([mo, 2 * T], F32, tag="pso")
                    if s == "ins":
                        nc.tensor.matmul(ps_o[:, 0:T], wt["sout_ins"][0:64, :],
                                         h[0:64, :], start=True, stop=True)
                        nc.tensor.matmul(ps_o[:, T:2 * T], wt["sout_ins"][64:128, :],
                                         h[64:128, :], start=True, stop=True)
                    else:
                        nc.tensor.matmul(ps_o[:, 0:T], wt["sout_" + s][:, 0:mo],
                                         h[:, 0:T], start=True, stop=True)
                        nc.tensor.matmul(ps_o[:, T:2 * T], wt["sout_" + s][:, 0:mo],
                                         h[:, T:2 * T], start=True, stop=True)
                    o_t = opool.tile([mo, 2 * T], F32, tag="o" + s)
                    nc.scalar.activation(o_t[:], ps_o[:], AF.Identity,
                                         bias=bout_t[s][:], scale=1.0)
                    lo, hi = OCH[s]
                    nc.sync.dma_start(out=out_d.ap()[lo:hi, c0:c0 + T],
                                      in_=o_t[:, 0:T])
                    nc.sync.dma_start(out=out_d.ap()[lo:hi, NG + c0:NG + c0 + T],
                                      in_=o_t[:, T:2 * T])

    nc.compile()
    return nc


def _prep_inputs(inputs):
    f32 = np.float32

    def wT(name):
        return np.ascontiguousarray(np.asarray(inputs[name], f32).T)

    m = {}
    m["sin_co"] = np.concatenate([np.concatenate([wT("cls_Win"), wT("obj_Win")], 1)] * 2, 0)
    m["sin_bp"] = np.concatenate([np.concatenate([wT("box_Win"), wT("pos_Win")], 1)] * 2, 0)
    m["sin_ins"] = _bd(wT("ins_Win"), wT("ins_Win"))
    for l in (1, 2):
        m[f"sl{l}_co"] = _bd(np.asarray(inputs["cls_Wh"][l - 1], f32).T,
                             np.asarray(inputs["obj_Wh"][l - 1], f32).T)
        m[f"sl{l}_bp"] = _bd(np.asarray(inputs["box_Wh"][l - 1], f32).T,
                             np.asarray(inputs["pos_Wh"][l - 1], f32).T)
        m[f"sl{l}_ins"] = _bd(np.asarray(inputs["ins_Wh"][l - 1], f32).T,
                              np.asarray(inputs["ins_Wh"][l - 1], f32).T)
    m["sout_co"] = _bd(wT("cls_Wout"), wT("obj_Wout"))        # [128, 83]
    m["sout_bp"] = _bd(wT("box_Wout"), wT("pos_Wout"))        # [128, 68]
    m["sout_ins"] = np.concatenate([wT("ins_Wout")] * 2, 0)   # [128, 128]

    def col(v):
        return np.ascontiguousarray(np.asarray(v, f32).reshape(-1, 1))

    m["bin_co"] = col(np.concatenate([inputs["cls_bin"], inputs["obj_bin"]]))
    m["bin_bp"] = col(np.concatenate([inputs["box_bin"], inputs["pos_bin"]]))
    m["bin_ins"] = col(np.concatenate([inputs["ins_bin"]] * 2))
    for l in (1, 2):
        m[f"bh{l}_co"] = col(np.concatenate([inputs["cls_bh"][l - 1], inputs["obj_bh"][l - 1]]))
        m[f"bh{l}_bp"] = col(np.concatenate([inputs["box_bh"][l - 1], inputs["pos_bh"][l - 1]]))
        m[f"bh{l}_ins"] = col(np.concatenate([inputs["ins_bh"][l - 1]] * 2))
    m["bout_co"] = col(np.concatenate([inputs["cls_bout"], inputs["obj_bout"]]))
    m["bout_bp"] = col(np.concatenate([inputs["box_bout"], inputs["pos_bout"]]))
    m["bout_ins"] = col(inputs["ins_bout"])

    m = {k: np.ascontiguousarray(v.astype(f32)) for k, v in m.items()}

    fast = all(not np.any(m[k]) for k in
               ["bh1_co", "bh1_bp", "bh1_ins", "bh2_co", "bh2_bp", "bh2_ins"])

    x = np.asarray(inputs["x"], f32)
    in_maps = []
    for c in range(NCORES):
        b, hh = c // 2, c % 2
        xs = x[b, :, hh * 128:(hh + 1) * 128, :].reshape(64, NPX)
        xsr = np.ascontiguousarray(
            np.concatenate([xs[:, :NG], xs[:, NG:]], axis=0))   # [128, NG]
        in_maps.append({**m, "xs": xsr})
    return in_maps, fast


def kernel(**inputs) -> np.ndarray:
    global _last_results
    in_maps, fast = _prep_inputs(inputs)
    if fast not in _cache:
        _cache[fast] = _build(fast)
    nc = _cache[fast]
    res = run_bass_kernel_spmd(nc, in_maps, core_ids=list(range(NCORES)))
    _last_results = res

    out = np.empty((B, OD, H, W), np.float32)
    for c in range(NCORES):
        b, hh = c // 2, c % 2
        out[b, :, hh * 128:(hh + 1) * 128, :] = \
            res.results[c]["out"].reshape(OD, 128, W)
    return out
